# revision 7
# baseline (speedup 1.0000x reference)
"""Trainium2 Bass kernel for nn_DARPGREATLayer (gnn_message_passing).

Strategy: data-parallel over B (8 batches -> 8 NeuronCores). Each core
computes one batch independently; no collectives.

Per-core pipeline (b = batch):
  Phase A (attention over edges, bf16 matmuls / fp32 accum):
    - edge[b] loaded in 10 tiles of (16 i) x (160 j) x (128 d), rows
      ordered j-major (row = j*16 + il), cast fp32->bf16 during SWDGE DMA,
      then xbar-transposed to edgeT [d=128, 2560 cols].
    - q/k/v projections as natural-layout matmuls (lhsT = edgeT slab,
      rhs = W) -> PSUM [rows, d].
    - scores s8[row,h] = sum_e q*k via DVE mult + segmented reduce.
    - p8 = exp(0.25*s8) on ACT (softmax max-subtraction skipped; scores
      are bounded ~|s|<1 so exp is safe; matches reference exactly).
    - tmp[row,(h,e)] = v * p8 (broadcast over e).
    - per-slab selector matmuls reduce 16-row groups on the PE:
        in-attn : sum over il (rows with same j)  -> agg_in  [d,160j]
        out-attn: sum over j  (rows with same il) -> agg_out [d,160i]
      l (softmax denominators) via the same trick on p8.
    - normalize: out = agg * (1/l) broadcast via mask matmul.
  Phase B (node path, fp32, transposed layout [d, n]):
    upd, LN, FFN, LN. LayerNorm over d done with ones-matmul row sums
    (transposed layout => g/be are per-partition scalars).
  Phase C (edge_out = xWi[i,:] + xWj[j,:] + b_edge):
    natural layout [i, (j,d)] built on the PE: identity-matmul broadcasts
    xWi over j; K=1 ones-matmul adds replicated xWj rows (bf16 operands,
    fp32 accumulation), ACT evacuates PSUM -> SBUF, DMA to HBM.
"""

import os
import sys

sys.path.insert(0, "/opt/trn_rl_repo")

import numpy as np
import ml_dtypes

import concourse.bass as bass
import concourse.tile as tile
from concourse import mybir
from concourse.bass_utils import run_bass_kernel_spmd
from concourse.vector_clock import ScopedClock

F32 = mybir.dt.float32
BF16 = mybir.dt.bfloat16
AP = bass.AP

B, N, D, H, FF = 8, 160, 128, 8, 512
DH = 16
SCALE = 0.25  # 1/sqrt(DH)
EPS = 1e-5

TI = 16          # i-rows per tile
NT = N // TI     # 10 tiles
COLS = N * TI    # 2560 columns per tile (row-major: col = j*TI + il)
NSLAB = COLS // 128  # 20 slabs per tile
GS = 4           # slabs per psum group
NG = NSLAB // GS  # 5 groups per tile

NCORES = 8

_BUILD_CACHE = {}

# ---------------------------------------------------------------------------
# patch: this walrus build rejects >1 sync waits on one CTRL instruction.
# Split the Tile tail-drain's accumulated sem waits into single-wait NOPs.
_PATCHED = False


def _patch_tile_drain():
    global _PATCHED
    if _PATCHED:
        return
    import bass_rust as _br

    def _patched_drain_and_barrier(self, tick_clock, wait_clock):
        drain_inst = self.nc.sync.drain()
        wait_clock.add_sem_waits(
            drain_inst.ins, ScopedClock({None: tick_clock.global_clock})
        )
        waits = list(drain_inst.ins.sync_info.on_wait)
        if len(waits) > 1:
            drain_inst.ins.sync_info.on_wait = []
            for w in waits:
                n = self.nc.sync.nop()
                n.ins.sync_info = _br.SyncInfo(on_wait=[w], on_update=[])
        self.nc.all_engine_barrier()
        assert self.sems is not None
        popped = self.nc._tile_sem_poison_stack.pop()
        assert popped is self._sem_poison
        self.nc.clear_and_free_semaphores(list(self.sems.allocated().values()))
        self.nc.all_engine_barrier()

    tile.TileContext._drain_and_barrier = _patched_drain_and_barrier
    _PATCHED = True


# ---------------------------------------------------------------------------


def _ap(t, offset, dims):
    """Raw AP on tensor handle t: dims = [[stride_elems, count], ...]."""
    return AP(t, offset, dims)


def _build_program(debug=False):
    """Build the per-core Bass program. SPMD: same program all cores."""
    _patch_tile_drain()
    nc = bass.Bass()

    dp = nc.declare_dram_parameter

    # --- inputs (per-core slices + replicated weights/constants)
    node_in = dp("node_in", [N, D], F32, isOutput=False)
    edge_in = dp("edge_in", [N * N, D], F32, isOutput=False)

    w_attn = dp("w_attn", [6, D, D], BF16, isOutput=False)  # q/k/v in, q/k/v out
    w_pi = dp("w_pi", [D, D], F32, isOutput=False)
    w_po = dp("w_po", [D, D], F32, isOutput=False)
    w_f1 = dp("w_f1", [D, FF], BF16, isOutput=False)
    w_f2 = dp("w_f2", [FF, D], BF16, isOutput=False)
    w_i = dp("w_i", [D, D], F32, isOutput=False)
    w_j = dp("w_j", [D, D], F32, isOutput=False)
    # per-partition scalar params, packed: columns =
    # 0:b_proj_sum 1:g1 2:be1 3:b_f2 4:g2 5:be2 6:b_edge 7:bv_in 8:bv_out
    pcol = dp("pcol", [D, 9], F32, isOutput=False)
    b_f1 = dp("b_f1", [D, 4], F32, isOutput=False)  # b_f1 chunks

    sel_j = dp("sel_j", [128, 8], BF16, isOutput=False)
    sel_i = dp("sel_i", [128, TI], BF16, isOutput=False)
    mask8 = dp("mask8", [8, 128], BF16, isOutput=False)
    ident_bf = dp("ident_bf", [128, 128], BF16, isOutput=False)
    ident_f32 = dp("ident_f32", [128, 128], F32, isOutput=False)
    ones_row_bf = dp("ones_row_bf", [1, 128], BF16, isOutput=False)
    ones_row_f32 = dp("ones_row_f32", [1, 128], F32, isOutput=False)
    wmean = dp("wmean", [128, 1], F32, isOutput=False)

    # --- outputs
    x_out = dp("x_out", [N, D], F32, isOutput=True)
    edge_out = dp("edge_out", [N * N, D], F32, isOutput=True)
    dbg = None
    if debug:
        dbg = dp("dbg", [128, 4096], F32, isOutput=True)

    with tile.TileContext(nc) as tc:
        _emit(nc, tc, locals())
    _split_excess_waits(nc)
    return nc


def _split_excess_waits(nc, max_waits=1):
    """This walrus build rejects multi-sem-wait instructions at codegen
    (setupSyncWait 'Too many sync wait commands'). Move excess waits onto
    same-engine NOPs inserted just before the instruction."""
    import bass_rust as _br

    for bb_wrap in nc.bb_map.values():
        bb = bb_wrap.bb
        insts = bb.instructions
        out = []
        for inst in insts:
            si = inst.sync_info
            if si is not None and len(si.on_wait) > max_waits:
                waits = list(si.on_wait)
                keep = waits[:max_waits]
                extra = waits[max_waits:]
                for w in extra:
                    nop = mybir.InstNoOp(
                        name=nc.get_next_instruction_name(), ins=[], outs=[]
                    )
                    nop.engine = inst.engine
                    nop.sync_info = _br.SyncInfo(on_wait=[w], on_update=[])
                    out.append(nop)
                inst.sync_info = _br.SyncInfo(
                    on_wait=keep, on_update=list(si.on_update)
                )
            out.append(inst)
        bb.instructions = out


def _emit(nc, tc, io):
    node_in, edge_in = io["node_in"], io["edge_in"]
    w_attn, w_pi, w_po = io["w_attn"], io["w_pi"], io["w_po"]
    w_f1, w_f2, w_i, w_j = io["w_f1"], io["w_f2"], io["w_i"], io["w_j"]
    pcol, b_f1 = io["pcol"], io["b_f1"]
    sel_j, sel_i, mask8 = io["sel_j"], io["sel_i"], io["mask8"]
    ident_bf, ident_f32 = io["ident_bf"], io["ident_f32"]
    ones_row_bf, ones_row_f32 = io["ones_row_bf"], io["ones_row_f32"]
    wmean = io["wmean"]
    x_out, edge_out, dbg = io["x_out"], io["edge_out"], io["dbg"]

    from contextlib import ExitStack

    ctx = ExitStack()
    with ctx:
        # ---- persistent SBUF constants
        cpool = ctx.enter_context(tc.tile_pool(name="consts", bufs=1))

        def load_const(src, shape, dtype, name):
            t = cpool.tile(shape, dtype, tag=name)
            nc.sync.dma_start(out=t[:, :], in_=src[:, :])
            return t

        wq_in = cpool.tile([D, D], BF16, tag="wq_in")
        nc.sync.dma_start(out=wq_in[:, :], in_=w_attn[0, :, :])
        wk_in = cpool.tile([D, D], BF16, tag="wk_in")
        nc.sync.dma_start(out=wk_in[:, :], in_=w_attn[1, :, :])
        wv_in = cpool.tile([D, D], BF16, tag="wv_in")
        nc.sync.dma_start(out=wv_in[:, :], in_=w_attn[2, :, :])
        wq_out = cpool.tile([D, D], BF16, tag="wq_out")
        nc.sync.dma_start(out=wq_out[:, :], in_=w_attn[3, :, :])
        wk_out = cpool.tile([D, D], BF16, tag="wk_out")
        nc.sync.dma_start(out=wk_out[:, :], in_=w_attn[4, :, :])
        wv_out = cpool.tile([D, D], BF16, tag="wv_out")
        nc.sync.dma_start(out=wv_out[:, :], in_=w_attn[5, :, :])

        selj_sb = load_const(sel_j, [128, 8], BF16, "selj")
        seli_sb = load_const(sel_i, [128, TI], BF16, "seli")
        mask8_sb = load_const(mask8, [8, 128], BF16, "mask8")
        identb_sb = load_const(ident_bf, [128, 128], BF16, "identb")
        identf_sb = load_const(ident_f32, [128, 128], F32, "identf")
        onesb_sb = load_const(ones_row_bf, [1, 128], BF16, "onesb")
        onesf_sb = load_const(ones_row_f32, [1, 128], F32, "onesf")
        wmean_sb = load_const(wmean, [128, 1], F32, "wmean")
        pcol_sb = load_const(pcol, [D, 9], F32, "pcol")
        bf1_sb = load_const(b_f1, [D, 4], F32, "bf1")

        # =================================================================
        # Phase A: edge attention
        # =================================================================
        apool = ctx.enter_context(tc.tile_pool(name="aphase", bufs=1))
        # persistent accumulators / per-tile tensors
        # out_in/out_out finals (transposed [d, n])
        out_inT = apool.tile([128, N], F32, tag="out_inT")
        out_outT = apool.tile([128, N], F32, tag="out_outT")

        with (
            tc.tile_pool(name="edget", bufs=2) as epool,
            tc.tile_pool(name="stage", bufs=2) as spool,
            tc.tile_pool(name="work", bufs=3) as wpool,
            tc.tile_pool(name="ps_q", bufs=1, space="PSUM") as pq,
            tc.tile_pool(name="ps_k", bufs=1, space="PSUM") as pk,
            tc.tile_pool(name="ps_v", bufs=2, space="PSUM") as pv,
            tc.tile_pool(name="ps_agg", bufs=1, space="PSUM") as pagg,
            tc.tile_pool(name="ps_l", bufs=1, space="PSUM") as pl,
        ):
            agg_ps = pagg.tile([128, 2 * N], F32, tag="agg")  # in | out
            l_ps = pl.tile([16, 2 * N], F32, tag="l")  # rows 0:8 in, 8:16 out

            for t in range(NT):
                # ---- load tile: HBM fp32 (j-major rows) -> SBUF bf16 natural
                nat = spool.tile([128, NSLAB, 128], BF16, tag="nat")
                src = _ap(
                    edge_in,
                    t * TI * N * D,
                    [
                        [D, 8],            # jlo (partition hi)
                        [N * D, TI],       # il  (partition lo)
                        [8 * D, NSLAB],    # slab
                        [1, D],            # d
                    ],
                )
                nc.gpsimd.dma_start(out=nat[:, :, :], in_=src)

                # ---- xbar transpose slabs -> edgeT [d, cols]
                edgeT = epool.tile([128, COLS], BF16, tag="edgeT")
                for s in range(NSLAB):
                    nc.sync.dma_start(
                        out=edgeT[:, s * 128 : (s + 1) * 128],
                        in_=nat[:, s, :],
                        transpose=True,
                    )

                for a, (wq, wk, wv) in enumerate(
                    ((wq_in, wk_in, wv_in), (wq_out, wk_out, wv_out))
                ):
                    # pass 1: scores
                    s8 = wpool.tile([128, NSLAB, 8], F32, tag="s8")
                    for g in range(NG):
                        q_ps = pq.tile([128, GS * 128], F32, tag="q")
                        k_ps = pk.tile([128, GS * 128], F32, tag="k")
                        for sl in range(GS):
                            s = g * GS + sl
                            lhs = edgeT[:, s * 128 : (s + 1) * 128]
                            nc.tensor.matmul(
                                q_ps[:, sl * 128 : (sl + 1) * 128],
                                lhs, wq[:, :], start=True, stop=True,
                            )
                            nc.tensor.matmul(
                                k_ps[:, sl * 128 : (sl + 1) * 128],
                                lhs, wk[:, :], start=True, stop=True,
                            )
                        k_sb = wpool.tile([128, GS * 128], BF16, tag="k_sb")
                        nc.scalar.copy(k_sb[:, :], k_ps[:, :])
                        prod = wpool.tile([128, GS * 128], BF16, tag="prod")
                        nc.vector.tensor_mul(
                            prod[:, :], q_ps[:, :], k_sb[:, :]
                        )
                        nc.vector.reduce_sum(
                            s8[:, g * GS : (g + 1) * GS, :],
                            prod.rearrange(
                                "p (sh e) -> p sh e", e=DH
                            )[:, :, :],
                            axis=mybir.AxisListType.X,
                        )
                    # exp (scale folded)
                    p8 = wpool.tile([128, NSLAB, 8], BF16, tag="p8")
                    nc.scalar.activation(
                        p8[:, :, :], s8[:, :, :],
                        mybir.ActivationFunctionType.Exp,
                        scale=SCALE,
                    )
                    # pass 2: v, tmp, selector matmuls
                    for g in range(NG):
                        v_ps = pv.tile([128, GS * 128], F32, tag="v")
                        for sl in range(GS):
                            s = g * GS + sl
                            lhs = edgeT[:, s * 128 : (s + 1) * 128]
                            nc.tensor.matmul(
                                v_ps[:, sl * 128 : (sl + 1) * 128],
                                lhs, wv[:, :], start=True, stop=True,
                            )
                        tmp = wpool.tile([128, GS, 8, DH], BF16, tag="tmp")
                        p8b = p8[:, g * GS : (g + 1) * GS, :].unsqueeze(
                            3
                        ).broadcast_to([128, GS, 8, DH])
                        nc.vector.tensor_mul(
                            tmp[:, :, :, :],
                            v_ps.rearrange(
                                "p (s h e) -> p s h e", h=8, e=DH
                            )[:, :, :, :],
                            p8b,
                        )
                        for sl in range(GS):
                            s = g * GS + sl
                            tmp_slab = tmp.rearrange(
                                "p s h e -> p (s h e)"
                            )[:, sl * 128 : (sl + 1) * 128]
                            p8_slab = p8.rearrange(
                                "p s h -> p (s h)"
                            )[:, s * 8 : (s + 1) * 8]
                            if a == 0:
                                # in-attn: reduce over il within j-groups
                                nc.tensor.matmul(
                                    agg_ps[:, s * 8 : (s + 1) * 8],
                                    tmp_slab, selj_sb[:, :],
                                    start=(t == 0), stop=(t == NT - 1),
                                )
                                nc.tensor.matmul(
                                    l_ps[0:8, s * 8 : (s + 1) * 8],
                                    p8_slab, selj_sb[:, :],
                                    start=(t == 0), stop=(t == NT - 1),
                                )
                            else:
                                # out-attn: reduce over j within il-groups
                                nc.tensor.matmul(
                                    agg_ps[:, N + t * TI : N + (t + 1) * TI],
                                    tmp_slab, seli_sb[:, :],
                                    start=(s == 0), stop=(s == NSLAB - 1),
                                )
                                nc.tensor.matmul(
                                    l_ps[0:8, N + t * TI : N + (t + 1) * TI],
                                    p8_slab, seli_sb[:, :],
                                    start=(s == 0), stop=(s == NSLAB - 1),
                                )

            # ---- normalize:  out = agg * (1/l) broadcast over e
            for a in range(2):
                linv = wpool.tile([8, N], F32, tag="linv")
                nc.vector.reciprocal(linv[:, :], l_ps[0:8, a * N : (a + 1) * N])
                linv_bf = wpool.tile([8, N], BF16, tag="linv_bf")
                nc.vector.tensor_copy(linv_bf[:, :], linv[:, :])
                lb_ps = pq.tile([128, N], F32, tag="q")
                nc.tensor.matmul(
                    lb_ps[:, :], mask8_sb[:, :], linv_bf[:, :],
                    start=True, stop=True,
                )
                agg_sb = wpool.tile([128, N], F32, tag="agg_sb")
                nc.scalar.copy(agg_sb[:, :], agg_ps[:, a * N : (a + 1) * N])
                dst = out_inT if a == 0 else out_outT
                nc.vector.tensor_mul(
                    dst[:, :], agg_sb[:, :], lb_ps[:, :]
                )
                # + v bias (zero in practice; per-partition scalar)
                nc.vector.tensor_scalar_add(
                    dst[:, :], dst[:, :], pcol_sb[:, 7 + a : 8 + a]
                )

        # =================================================================
        # Phase B: node path (transposed layout [d, n], fp32)
        # =================================================================
        bpool = ctx.enter_context(tc.tile_pool(name="bphase", bufs=1))
        bps = ctx.enter_context(tc.tile_pool(name="ps_b", bufs=2, space="PSUM"))
        bps1 = ctx.enter_context(
            tc.tile_pool(name="ps_b1", bufs=1, space="PSUM")
        )

        wpi_sb = bpool.tile([D, D], F32, tag="wpi")
        nc.sync.dma_start(out=wpi_sb[:, :], in_=w_pi[:, :])
        wpo_sb = bpool.tile([D, D], F32, tag="wpo")
        nc.sync.dma_start(out=wpo_sb[:, :], in_=w_po[:, :])

        # nodeT via PE transpose
        node_sb0 = bpool.tile([128, D], F32, tag="node0")
        nc.sync.dma_start(out=node_sb0[:, :], in_=node_in[0:128, :])
        node_sb1 = bpool.tile([32, D], F32, tag="node1")
        nc.sync.dma_start(out=node_sb1[:, :], in_=node_in[128:160, :])
        ntp = bps.tile([128, N], F32, tag="bps")
        nc.tensor.transpose(ntp[:, 0:128], node_sb0[:, :], identf_sb[:, :])
        nc.tensor.transpose(ntp[:, 128:160], node_sb1[:, :], identf_sb[0:32, 0:32])
        nodeT = bpool.tile([128, N], F32, tag="nodeT")
        nc.scalar.copy(nodeT[:, :], ntp[:, :])

        # updT = Wpi.T @ out_inT + Wpo.T @ out_outT  (+ b_proj via scalar)
        upd_ps = bps.tile([128, N], F32, tag="bps")
        nc.tensor.matmul(upd_ps[:, :], wpi_sb[:, :], out_inT[:, :],
                         start=True, stop=False)
        nc.tensor.matmul(upd_ps[:, :], wpo_sb[:, :], out_outT[:, :],
                         start=False, stop=True)
        pre1 = bpool.tile([128, N], F32, tag="pre1")
        nc.vector.scalar_tensor_tensor(
            pre1[:, :], upd_ps[:, :], pcol_sb[:, 0:1], nodeT[:, :],
            op0=mybir.AluOpType.add, op1=mybir.AluOpType.add,
        )

        def layer_norm_T(xT, g_col, be_col, out_tag):
            """LN over partition dim (d) of [128, N] via ones-matmuls."""
            mean_ps = bps1.tile([1, 2 * N], F32, tag="mean")
            nc.tensor.matmul(mean_ps[0:1, 0:N], wmean_sb[:, :], xT[:, :],
                             start=True, stop=True)
            sq = bpool.tile([128, N], F32, tag="sq")
            nc.vector.tensor_mul(sq[:, :], xT[:, :], xT[:, :])
            nc.tensor.matmul(mean_ps[0:1, N : 2 * N], wmean_sb[:, :],
                             sq[:, :], start=True, stop=True)
            mean_sb = bpool.tile([1, 2 * N], F32, tag="mean_sb")
            nc.vector.tensor_copy(mean_sb[0:1, :], mean_ps[0:1, :])
            # rstd = 1/sqrt(msq - mean^2 + eps)
            var = bpool.tile([1, N], F32, tag="var")
            nc.vector.scalar_tensor_tensor(
                var[0:1, :],
                mean_sb[0:1, 0:N], 0.0, mean_sb[0:1, 0:N],
                op0=mybir.AluOpType.add, op1=mybir.AluOpType.mult,
            )  # mean^2
            nc.vector.tensor_sub(var[0:1, :], mean_sb[0:1, N : 2 * N],
                                 var[0:1, :])
            nc.vector.tensor_scalar_add(var[0:1, :], var[0:1, :], float(EPS))
            vs = bpool.tile([1, N], F32, tag="vs")
            nc.scalar.activation(
                vs[0:1, :], var[0:1, :],
                mybir.ActivationFunctionType.Sqrt,
            )
            rstd = bpool.tile([1, N], F32, tag="rstd")
            nc.vector.reciprocal(rstd[0:1, :], vs[0:1, :])
            # broadcast mean, rstd to 128 partitions
            mb_ps = bps.tile([128, N], F32, tag="bps")
            nc.tensor.matmul(mb_ps[:, :], onesf_sb[:, :], mean_sb[0:1, 0:N],
                             start=True, stop=True)
            rb_ps = bps.tile([128, N], F32, tag="bps")
            nc.tensor.matmul(rb_ps[:, :], onesf_sb[:, :], rstd[0:1, :],
                             start=True, stop=True)
            cen = bpool.tile([128, N], F32, tag="cen")
            nc.vector.tensor_sub(cen[:, :], xT[:, :], mb_ps[:, :])
            xh = bpool.tile([128, N], F32, tag=out_tag)
            nc.vector.tensor_mul(xh[:, :], cen[:, :], rb_ps[:, :])
            nc.vector.tensor_scalar(
                xh[:, :], xh[:, :], g_col, be_col,
                op0=mybir.AluOpType.mult, op1=mybir.AluOpType.add,
            )
            return xh

        x1 = layer_norm_T(pre1, pcol_sb[:, 1:2], pcol_sb[:, 2:3], "x1")

        # FFN
        x1_bf = bpool.tile([128, N], BF16, tag="x1bf")
        nc.vector.tensor_copy(x1_bf[:, :], x1[:, :])
        wf1_sb = bpool.tile([D, FF], BF16, tag="wf1")
        nc.sync.dma_start(out=wf1_sb[:, :], in_=w_f1[:, :])
        wf2_sb = bpool.tile([128, 4 * D], BF16, tag="wf2")
        nc.sync.dma_start(
            out=wf2_sb.rearrange("p (c d) -> p c d", c=4)[:, :, :],
            in_=w_f2[:, :].rearrange("(c p) d -> p c d", c=4)[:, :, :],
        )
        hT = bpool.tile([128, 4 * N], BF16, tag="hT")
        for c in range(4):
            h_ps = bps.tile([128, N], F32, tag="bps")
            nc.tensor.matmul(
                h_ps[:, :], wf1_sb[:, c * 128 : (c + 1) * 128], x1_bf[:, :],
                start=True, stop=True,
            )
            nc.scalar.activation(
                hT[:, c * N : (c + 1) * N], h_ps[:, :],
                mybir.ActivationFunctionType.Relu,
                bias=bf1_sb[:, c : c + 1],
            )
        f_ps = bps.tile([128, N], F32, tag="bps")
        for c in range(4):
            nc.tensor.matmul(
                f_ps[:, :], wf2_sb[:, c * 128 : (c + 1) * 128],
                hT[:, c * N : (c + 1) * N],
                start=(c == 0), stop=(c == 3),
            )
        pre2 = bpool.tile([128, N], F32, tag="pre2")
        nc.vector.scalar_tensor_tensor(
            pre2[:, :], f_ps[:, :], pcol_sb[:, 3:4], x1[:, :],
            op0=mybir.AluOpType.add, op1=mybir.AluOpType.add,
        )
        x2 = layer_norm_T(pre2, pcol_sb[:, 4:5], pcol_sb[:, 5:6], "x2")

        # x output: transpose back to [n, d] and store
        xo_ps = bps.tile([128, N], F32, tag="bps")
        nc.tensor.transpose(xo_ps[:, 0:128], x2[:, 0:128], identf_sb[:, :])
        xnat0 = bpool.tile([128, D], F32, tag="xnat0")
        nc.scalar.copy(xnat0[:, :], xo_ps[:, 0:128])
        nc.sync.dma_start(out=x_out[0:128, :], in_=xnat0[:, :])
        xo1_ps = bps.tile([128, N], F32, tag="bps")
        nc.tensor.transpose(xo1_ps[0:32, 0:128], x2[:, 128:160],
                            identf_sb[:, :])
        xnat1 = bpool.tile([32, D], F32, tag="xnat1")
        nc.scalar.copy(xnat1[:, :], xo1_ps[0:32, 0:128])
        nc.sync.dma_start(out=x_out[128:160, :], in_=xnat1[:, :])

        if dbg is not None:
            nc.sync.dma_start(out=dbg[:, 0:160], in_=out_inT[:, :])
            nc.sync.dma_start(out=dbg[:, 256:416], in_=out_outT[:, :])
            nc.sync.dma_start(out=dbg[:, 512:672], in_=x2[:, :])
            nc.sync.dma_start(out=dbg[:, 768:928], in_=pre1[:, :])

        # =================================================================
        # Phase C: edge_out[i, j, :] = xWi[i,:] + xWj[j,:] + b_edge
        # =================================================================
        wi_sb = bpool.tile([D, D], F32, tag="wi")
        nc.sync.dma_start(out=wi_sb[:, :], in_=w_i[:, :])
        wj_sb = bpool.tile([D, D], F32, tag="wj")
        nc.sync.dma_start(out=wj_sb[:, :], in_=w_j[:, :])

        xwi_ps = bps.tile([128, N], F32, tag="bps")
        nc.tensor.matmul(xwi_ps[:, :], wi_sb[:, :], x2[:, :],
                         start=True, stop=True)
        xwiT_bf = bpool.tile([128, N], BF16, tag="xwiT")
        nc.vector.tensor_copy(xwiT_bf[:, :], xwi_ps[:, :])

        xwj_ps = bps.tile([128, N], F32, tag="bps")
        nc.tensor.matmul(xwj_ps[:, :], wj_sb[:, :], x2[:, :],
                         start=True, stop=True)
        xwjT_bf = bpool.tile([128, N], BF16, tag="xwjT")
        # + b_edge folded here (per-partition scalar in transposed layout)
        nc.vector.tensor_scalar_add(
            xwjT_bf[:, :], xwj_ps[:, :], pcol_sb[:, 6:7]
        )
        # transpose xWj to natural bf16 and flatten to one partition row
        xwjn_ps = bps.tile([128, N], BF16, tag="bps_bf")
        nc.tensor.transpose(xwjn_ps[:, 0:128], xwjT_bf[:, 0:128],
                            identb_sb[:, :])
        xwjn0 = bpool.tile([128, D], BF16, tag="xwjn0")
        nc.scalar.copy(xwjn0[:, :], xwjn_ps[:, 0:128])
        xwjn1_ps = bps.tile([128, N], BF16, tag="bps_bf")
        nc.tensor.transpose(xwjn1_ps[0:32, 0:128], xwjT_bf[:, 128:160],
                            identb_sb[:, :])
        xwjn1 = bpool.tile([32, D], BF16, tag="xwjn1")
        nc.scalar.copy(xwjn1[:, :], xwjn1_ps[0:32, 0:128])
        flat = bpool.tile([1, N * D], BF16, tag="flat")
        nc.sync.dma_start(
            out=flat.rearrange("o (j d) -> o j d", d=D)[:, 0:128, :],
            in_=xwjn0[:, :],
        )
        nc.sync.dma_start(
            out=flat.rearrange("o (j d) -> o j d", d=D)[:, 128:160, :],
            in_=xwjn1[:, :],
        )

        CJ = 4          # j's per chunk
        CW = CJ * D     # 512 cols per chunk
        with (
            tc.tile_pool(name="cphase", bufs=3) as cpool2,
            tc.tile_pool(name="ps_c", bufs=3, space="PSUM") as pc,
        ):
            for islab, (i0, ni) in enumerate(((0, 128), (128, 32))):
                for jc in range(N // CJ):
                    eo_ps = pc.tile([128, CW], F32, tag="eo")
                    ident_b = identb_sb.unsqueeze(1).broadcast_to(
                        [128, CJ, 128]
                    )
                    nc.tensor.matmul(
                        eo_ps[0:ni, :],
                        xwiT_bf[:, i0 : i0 + ni],
                        ident_b,
                        start=True, stop=False,
                    )
                    nc.tensor.matmul(
                        eo_ps[0:ni, :],
                        onesb_sb[0:1, 0:ni],
                        flat[0:1, jc * CW : (jc + 1) * CW],
                        start=False, stop=True,
                    )
                    eo_sb = cpool2.tile([128, CW], F32, tag="eo_sb")
                    nc.scalar.copy(eo_sb[0:ni, :], eo_ps[0:ni, :])
                    dst = _ap(
                        edge_out,
                        (i0 * N + jc * CJ) * D,
                        [[N * D, ni], [1, CW]],
                    )
                    nc.sync.dma_start(out=dst, in_=eo_sb[0:ni, :])


# ---------------------------------------------------------------------------
# host side
# ---------------------------------------------------------------------------


def _prep_inputs(inputs):
    """Build the per-core in_maps from full inputs."""
    f32 = np.float32
    bf = ml_dtypes.bfloat16

    node = np.asarray(inputs["node_emb"], f32)
    edge = np.asarray(inputs["edge_emb"], f32)

    w_attn = np.stack(
        [
            np.asarray(inputs["W_q_in"], f32),
            np.asarray(inputs["W_k_in"], f32),
            np.asarray(inputs["W_v_in"], f32),
            np.asarray(inputs["W_q_out"], f32),
            np.asarray(inputs["W_k_out"], f32),
            np.asarray(inputs["W_v_out"], f32),
        ]
    ).astype(bf)

    W_edge = np.asarray(inputs["W_edge"], f32)
    pcol = np.zeros((D, 9), f32)
    pcol[:, 0] = np.asarray(inputs["b_proj_in"], f32) + np.asarray(
        inputs["b_proj_out"], f32
    )
    pcol[:, 1] = np.asarray(inputs["g_attn"], f32)
    pcol[:, 2] = np.asarray(inputs["be_attn"], f32)
    pcol[:, 3] = np.asarray(inputs["b_f2"], f32)
    pcol[:, 4] = np.asarray(inputs["g_ffn"], f32)
    pcol[:, 5] = np.asarray(inputs["be_ffn"], f32)
    pcol[:, 6] = np.asarray(inputs["b_edge"], f32)
    # v biases (columns 7,8) - must be zero for this kernel's fast path
    bv_in = np.asarray(inputs["b_v_in"], f32)
    bv_out = np.asarray(inputs["b_v_out"], f32)
    pcol[:, 7] = bv_in
    pcol[:, 8] = bv_out

    b_f1 = np.asarray(inputs["b_f1"], f32).reshape(4, D).T.copy()

    rows = np.arange(128)
    sel_j_np = (rows[:, None] // TI == np.arange(8)[None, :]).astype(bf)
    sel_i_np = (rows[:, None] % TI == np.arange(TI)[None, :]).astype(bf)
    mask8_np = (np.arange(128)[None, :] // DH == np.arange(8)[:, None]).astype(
        bf
    )
    ident = np.eye(128, dtype=f32)

    shared = {
        "w_attn": w_attn,
        "w_pi": np.asarray(inputs["W_proj_in"], f32),
        "w_po": np.asarray(inputs["W_proj_out"], f32),
        "w_f1": np.asarray(inputs["W_f1"], f32).astype(bf),
        "w_f2": np.asarray(inputs["W_f2"], f32).astype(bf),
        "w_i": W_edge[:D].copy(),
        "w_j": W_edge[D:].copy(),
        "pcol": pcol,
        "b_f1": b_f1,
        "sel_j": sel_j_np,
        "sel_i": sel_i_np,
        "mask8": mask8_np,
        "ident_bf": ident.astype(bf),
        "ident_f32": ident,
        "ones_row_bf": np.ones((1, 128), bf),
        "ones_row_f32": np.ones((1, 128), f32),
        "wmean": np.full((128, 1), 1.0 / D, f32),
    }
    in_maps = []
    for b in range(NCORES):
        m = dict(shared)
        m["node_in"] = node[b].copy()
        m["edge_in"] = edge[b].reshape(N * N, D).copy()
        in_maps.append(m)
    return in_maps


def kernel(**inputs):
    debug = bool(os.environ.get("KERNEL_DEBUG"))
    key = ("prog", debug)
    if key not in _BUILD_CACHE:
        _BUILD_CACHE[key] = _build_program(debug=debug)
    nc = _BUILD_CACHE[key]

    in_maps = _prep_inputs(inputs)
    res = run_bass_kernel_spmd(nc, in_maps, list(range(NCORES)))
    x = np.stack([res.results[b]["x_out"] for b in range(NCORES)])
    eo = np.stack(
        [res.results[b]["edge_out"].reshape(N, N, D) for b in range(NCORES)]
    )
    if debug:
        kernel.dbg = np.stack([res.results[b]["dbg"] for b in range(NCORES)])
    return x.astype(np.float32), eo.astype(np.float32)
